# revision 4
# baseline (speedup 1.0000x reference)
"""Trainium2 Bass kernel: Brevitas-style int4 fake-quant Conv2d (3x3, pad 1).

reference:
    wq = fake_quant_per_channel(w)          # per-O-channel int4 scale
    out = conv2d(x, wq, NCHW/OIHW, pad 1)

Strategy:
  * Host: per-channel abs-max quant -> integer weights q in [-7, 7]
    (exactly representable in bf16) + per-channel f32 scale.  The scale is
    applied on-device during the PSUM->SBUF drain, so the matmuls run on
    exact integer weights.
  * Device: data-parallel over batch (4 images per core x 8 cores).
    Conv lowered to implicit GEMM: for each of the 9 taps,
    out[o, p] += q[o, :, tap] @ x_shifted[:, p], accumulated in PSUM over
    (2 k-tiles x 9 taps x 2 passes).  x is stored zero-padded (58x58) in
    SBUF so every tap is a constant offset into a contiguous buffer.
  * Accuracy: x is split hi/lo into two bf16 tensors (hi = bf16(x),
    lo = bf16(x - hi)); both passes accumulate into the same PSUM bank.
    This carries ~16-17 mantissa bits of x through the PE array -> rel
    err ~1e-6 vs the f32 reference, at 2 bf16 matmul passes (half the
    cost of native fp32 matmul, which runs at 1/4 rate).
"""

import os
import sys
from contextlib import ExitStack

for _p in ("/opt/trn_rl_repo", "/root/.axon_site/_ro/trn_rl_repo"):
    if os.path.isdir(_p) and _p not in sys.path:
        sys.path.insert(0, _p)

import numpy as np
import ml_dtypes

import concourse.bass as bass  # noqa: F401  (bass types used via tile/bacc)
import concourse.mybir as mybir
import concourse.tile as tile
from concourse import bacc
from concourse.bass_utils import run_bass_kernel_spmd

F32 = mybir.dt.float32
BF16 = mybir.dt.bfloat16

# Problem shapes (hardcoded per contract).
N, C, H, W = 32, 256, 56, 56
O, KH, KW = 256, 3, 3
CORES = 8
NPC = N // CORES  # images per core

QMAX = 7.0
SCALING_MIN_VAL = 2e-16


def build_nc(npc=NPC, c=C, h=H, w=W, o=O, two_pass=True, ch_rows=8):
    """Build the per-core Bass program (SPMD: same program on all cores).

    DRAM I/O (per core):
      x     [npc, c, h, w]  f32   batch shard
      wq    [128, 9*KT*OT*128] bf16  integer weights, layout [i, (tap,kt,ot,o)]
      scale [128, OT]       f32   per-out-channel scale, [o_local, ot]
      out   [npc, o, h, w]  f32
    """
    KT = c // 128
    OT = o // 128
    Hp, Wp = h + 2, w + 2
    PADN = Hp * Wp + 2          # +1 guard cell at each end
    assert h % ch_rows == 0
    n_chunks = h // ch_rows
    rhs_len = ch_rows * Wp
    assert rhs_len <= 512       # one PSUM bank (f32)

    nc = bacc.Bacc("TRN2", target_bir_lowering=False, debug=False)
    x_d = nc.dram_tensor("x", [npc, c, h, w], F32, kind="ExternalInput").ap()
    w_d = nc.dram_tensor("wq", [128, 9 * KT * OT * 128], BF16,
                         kind="ExternalInput").ap()
    s_d = nc.dram_tensor("scale", [128, OT], F32, kind="ExternalInput").ap()
    out_d = nc.dram_tensor("out", [npc, o, h, w], F32,
                           kind="ExternalOutput").ap()

    n_passes = 2 if two_pass else 1

    with tile.TileContext(nc) as tc, ExitStack() as ctx:
        wpool = ctx.enter_context(tc.tile_pool(name="wpool", bufs=1))
        xpool = ctx.enter_context(tc.tile_pool(name="xpool", bufs=2))
        hpool = ctx.enter_context(tc.tile_pool(name="hpool", bufs=2))
        opool = ctx.enter_context(tc.tile_pool(name="opool", bufs=4))
        ppool = ctx.enter_context(tc.tile_pool(name="ppool",
                                               bufs=min(8, n_chunks + 1),
                                               space="PSUM"))

        w_sb = wpool.tile([128, 9 * KT * OT * 128], BF16)
        nc.sync.dma_start(w_sb[:, :], w_d[:, :])
        s_sb = wpool.tile([128, OT], F32)
        nc.sync.dma_start(s_sb[:, :], s_d[:, :])

        def pad_views(t):
            """(interior [128,h,w] view, border memset APs) of padded tile."""
            interior = (t[:, Wp + 2: Wp + 2 + h * Wp]
                        .rearrange("p (r c) -> p r c", c=Wp)[:, :, 0:w])
            m1 = t[:, 0: Wp + 2]
            m2 = t[:, PADN - (Wp + 2): PADN]
            m3 = (t[:, 2 * Wp: 2 * Wp + (Hp - 3) * Wp]
                  .rearrange("p (a b) -> p a b", b=Wp)[:, :, 0:2])
            return interior, (m1, m2, m3)

        for img in range(npc):
            pads = []  # [kt] -> list of padded bf16 tiles (hi[, lo])
            for kt in range(KT):
                xf = xpool.tile([128, h * w], F32, tag="xf")
                nc.sync.dma_start(
                    xf[:, :].rearrange("p (r c) -> p r c", c=w),
                    x_d[img, kt * 128:(kt + 1) * 128, :, :])
                xv = xf[:, :].rearrange("p (r c) -> p r c", c=w)

                hi = hpool.tile([128, PADN], BF16, tag=f"hi{kt}")
                hi_int, hi_borders = pad_views(hi)
                for m in hi_borders:
                    nc.vector.memset(m, 0.0)
                nc.vector.tensor_copy(hi_int, xv)
                ptiles = [hi]
                if two_pass:
                    lo = hpool.tile([128, PADN], BF16, tag=f"lo{kt}")
                    lo_int, lo_borders = pad_views(lo)
                    for m in lo_borders:
                        nc.vector.memset(m, 0.0)
                    # lo = bf16(x - f32(hi)); exact f32 subtract on DVE
                    nc.vector.tensor_sub(lo_int, xv, hi_int)
                    ptiles.append(lo)
                pads.append(ptiles)

            for ot in range(OT):
                psums = [ppool.tile([128, ch_rows, Wp], F32, tag="ps",
                                    name=f"ps{ci}")
                         for ci in range(n_chunks)]
                n_acc = KT * 9 * n_passes
                idx = 0
                for kt in range(KT):
                    for tap in range(9):
                        dh, dw = divmod(tap, 3)
                        wv = w_sb[:, ((tap * KT + kt) * OT + ot) * 128:
                                     ((tap * KT + kt) * OT + ot) * 128 + 128]
                        for pt in pads[kt]:
                            for ci in range(n_chunks):
                                s0 = (ci * ch_rows + dh) * Wp + dw
                                nc.tensor.matmul(
                                    psums[ci][:, :, :],
                                    wv,
                                    pt[:, s0: s0 + rhs_len],
                                    start=(idx == 0),
                                    stop=(idx == n_acc - 1),
                                )
                            idx += 1
                for ci in range(n_chunks):
                    ob = opool.tile([128, ch_rows, w], F32, tag="ob")
                    nc.scalar.mul(ob[:, :, :],
                                  psums[ci][:, :, 1: 1 + w],
                                  s_sb[:, ot: ot + 1])
                    nc.sync.dma_start(
                        out_d[img, ot * 128:(ot + 1) * 128,
                              ci * ch_rows:(ci + 1) * ch_rows, :],
                        ob[:, :, :])

    nc.compile()
    return nc


def quantize_weights(w):
    """Match reference fake-quant in f32: returns (q int-valued f32, scale)."""
    w = np.asarray(w, np.float32)
    amax = np.max(np.abs(w), axis=(1, 2, 3), keepdims=True).astype(np.float32)
    scale = np.maximum((amax / np.float32(QMAX)).astype(np.float32),
                       np.float32(SCALING_MIN_VAL)).astype(np.float32)
    q = np.clip(np.rint((w / scale).astype(np.float32)),
                -QMAX, QMAX).astype(np.float32)
    return q, scale.reshape(-1)


def pack_weights(q, o=O, c=C):
    """q [O,C,3,3] int-valued -> [128, 9*KT*OT*128] bf16, [i,(tap,kt,ot,o)]."""
    KT = c // 128
    OT = o // 128
    w5 = q.reshape(OT, 128, KT, 128, KH, KW)       # [ot, ol, kt, i, kh, kw]
    w5 = w5.transpose(3, 4, 5, 2, 0, 1)            # [i, kh, kw, kt, ot, ol]
    w5 = np.ascontiguousarray(w5).reshape(128, 9 * KT * OT * 128)
    return w5.astype(ml_dtypes.bfloat16)


_nc_cache = {}


def kernel(x, w):
    x = np.ascontiguousarray(np.asarray(x, np.float32))
    w = np.asarray(w, np.float32)
    assert x.shape == (N, C, H, W) and w.shape == (O, C, KH, KW)

    q, scale = quantize_weights(w)
    w_host = pack_weights(q)
    s_host = np.ascontiguousarray(
        scale.reshape(O // 128, 128).T).astype(np.float32)  # [o_local, ot]

    if "nc" not in _nc_cache:
        _nc_cache["nc"] = build_nc()
    nc = _nc_cache["nc"]

    in_maps = [
        {"x": np.ascontiguousarray(x[cid * NPC:(cid + 1) * NPC]),
         "wq": w_host, "scale": s_host}
        for cid in range(CORES)
    ]
    res = run_bass_kernel_spmd(nc, in_maps, list(range(CORES)))
    return np.concatenate([res.results[cid]["out"] for cid in range(CORES)],
                          axis=0)


if __name__ == "__main__":
    rng = np.random.default_rng(0)
    x = rng.standard_normal((N, C, H, W), dtype=np.float32)
    w = rng.standard_normal((O, C, KH, KW), dtype=np.float32) * 0.05
    out = kernel(x, w)
    print("out", out.shape, out.dtype, float(np.abs(out).max()))


# revision 7
# speedup vs baseline: 1.8287x; 1.8287x over previous
"""Trainium2 Bass kernel: Brevitas-style int4 fake-quant Conv2d (3x3, pad 1).

reference:
    wq = fake_quant_per_channel(w)          # per-O-channel int4 scale
    out = conv2d(x, wq, NCHW/OIHW, pad 1)

Strategy:
  * Host: per-channel abs-max quant -> integer weights q in [-7, 7]
    (exactly representable in bf16) + per-channel f32 scale.  The scale is
    applied on-device during the PSUM->SBUF drain, so the matmuls run on
    exact integer weights.
  * Device: data-parallel over batch (4 images per core x 8 cores).
    Conv lowered to implicit GEMM: for each of the 9 taps,
    out[o, p] += q[o, :, tap] @ x_shifted[:, p], accumulated in PSUM over
    (2 k-tiles x 9 taps x 2 passes).  x is stored zero-padded (58x58) in
    SBUF so every tap is a constant offset into a contiguous buffer.
  * Accuracy: x is split hi/lo into two bf16 tensors (hi = bf16(x),
    lo = bf16(x - hi)); both passes accumulate into the same PSUM bank.
    This carries ~16-17 mantissa bits of x through the PE array -> rel
    err ~1e-6 vs the f32 reference, at 2 bf16 matmul passes (half the
    cost of native fp32 matmul, which runs at 1/4 rate).
"""

import os
import sys
from contextlib import ExitStack

for _p in ("/opt/trn_rl_repo", "/root/.axon_site/_ro/trn_rl_repo"):
    if os.path.isdir(_p) and _p not in sys.path:
        sys.path.insert(0, _p)

import numpy as np
import ml_dtypes

import concourse.bass as bass  # noqa: F401  (bass types used via tile/bacc)
import concourse.mybir as mybir
import concourse.tile as tile
from concourse import bacc
from concourse.bass_utils import run_bass_kernel_spmd

F32 = mybir.dt.float32
BF16 = mybir.dt.bfloat16
FP16 = mybir.dt.float16

# Problem shapes (hardcoded per contract).
N, C, H, W = 32, 256, 56, 56
O, KH, KW = 256, 3, 3
CORES = 8
NPC = N // CORES  # images per core

QMAX = 7.0
SCALING_MIN_VAL = 2e-16


def build_nc(npc=NPC, c=C, h=H, w=W, o=O, two_pass=True, ch_rows=8,
             dt=BF16, strided_rhs=False):
    """Build the per-core Bass program (SPMD: same program on all cores).

    DRAM I/O (per core):
      x     [npc, c, h, w]  f32   batch shard
      wq    [128, 9*KT*OT*128] bf16  integer weights, layout [i, (tap,kt,ot,o)]
      scale [128, OT]       f32   per-out-channel scale, [o_local, ot]
      out   [npc, o, h, w]  f32
    """
    KT = c // 128
    OT = o // 128
    Hp, Wp = h + 2, w + 2
    PADN = Hp * Wp + 2          # +1 guard cell at each end
    assert h % ch_rows == 0
    n_chunks = h // ch_rows
    rhs_len = ch_rows * Wp
    assert rhs_len <= 512       # one PSUM bank (f32)

    nc = bacc.Bacc("TRN2", target_bir_lowering=False, debug=False)
    x_d = nc.dram_tensor("x", [npc, c, h, w], F32, kind="ExternalInput").ap()
    w_d = nc.dram_tensor("wq", [128, 9 * KT * OT * 128], dt,
                         kind="ExternalInput").ap()
    s_d = nc.dram_tensor("scale", [128, OT], F32, kind="ExternalInput").ap()
    out_d = nc.dram_tensor("out", [npc, o, h, w], F32,
                           kind="ExternalOutput").ap()

    n_passes = 2 if two_pass else 1

    with tile.TileContext(nc) as tc, ExitStack() as ctx:
        wpool = ctx.enter_context(tc.tile_pool(name="wpool", bufs=1))
        xpool = ctx.enter_context(tc.tile_pool(name="xpool", bufs=2))
        hpool = ctx.enter_context(tc.tile_pool(name="hpool", bufs=2))
        opool = ctx.enter_context(tc.tile_pool(name="opool", bufs=4))
        ppool = ctx.enter_context(tc.tile_pool(name="ppool",
                                               bufs=min(8, n_chunks + 1),
                                               space="PSUM"))

        w_sb = wpool.tile([128, 9 * KT * OT * 128], dt)
        nc.sync.dma_start(w_sb[:, :], w_d[:, :])
        s_sb = wpool.tile([128, OT], F32)
        nc.sync.dma_start(s_sb[:, :], s_d[:, :])

        def pad_views(t):
            """(interior [128,h,w] view, border memset APs) of padded tile."""
            interior = (t[:, Wp + 2: Wp + 2 + h * Wp]
                        .rearrange("p (r c) -> p r c", c=Wp)[:, :, 0:w])
            m1 = t[:, 0: Wp + 2]
            m2 = t[:, PADN - (Wp + 2): PADN]
            m3 = (t[:, 2 * Wp: 2 * Wp + (Hp - 3) * Wp]
                  .rearrange("p (a b) -> p a b", b=Wp)[:, :, 0:2])
            return interior, (m1, m2, m3)

        for img in range(npc):
            pads = []  # [kt] -> list of padded bf16 tiles (hi[, lo])
            for kt in range(KT):
                xf = xpool.tile([128, h * w], F32, tag="xf")
                nc.sync.dma_start(
                    xf[:, :].rearrange("p (r c) -> p r c", c=w),
                    x_d[img, kt * 128:(kt + 1) * 128, :, :])
                xv = xf[:, :].rearrange("p (r c) -> p r c", c=w)

                hi = hpool.tile([128, PADN], dt, tag=f"hi{kt}")
                hi_int, hi_borders = pad_views(hi)
                for m in hi_borders:
                    nc.vector.memset(m, 0.0)
                nc.vector.tensor_copy(hi_int, xv)
                ptiles = [hi]
                if two_pass:
                    lo = hpool.tile([128, PADN], dt, tag=f"lo{kt}")
                    lo_int, lo_borders = pad_views(lo)
                    for m in lo_borders:
                        nc.vector.memset(m, 0.0)
                    # lo = bf16(x - f32(hi)); exact f32 subtract on DVE
                    nc.vector.tensor_sub(lo_int, xv, hi_int)
                    ptiles.append(lo)
                pads.append(ptiles)

            psum_w = w if strided_rhs else Wp
            for ot in range(OT):
                psums = [ppool.tile([128, ch_rows, psum_w], F32, tag="ps",
                                    name=f"ps{ci}")
                         for ci in range(n_chunks)]
                n_acc = KT * 9 * n_passes
                idx = 0
                for kt in range(KT):
                    for tap in range(9):
                        dh, dw = divmod(tap, 3)
                        wv = w_sb[:, ((tap * KT + kt) * OT + ot) * 128:
                                     ((tap * KT + kt) * OT + ot) * 128 + 128]
                        for pt in pads[kt]:
                            for ci in range(n_chunks):
                                s0 = (ci * ch_rows + dh) * Wp + dw
                                if strided_rhs:
                                    rhs = (pt[:, s0: s0 + rhs_len]
                                           .rearrange("p (r c) -> p r c",
                                                      c=Wp)[:, :, 0:w])
                                else:
                                    rhs = pt[:, s0: s0 + rhs_len]
                                nc.tensor.matmul(
                                    psums[ci][:, :, :],
                                    wv,
                                    rhs,
                                    start=(idx == 0),
                                    stop=(idx == n_acc - 1),
                                )
                            idx += 1
                for ci in range(n_chunks):
                    ob = opool.tile([128, ch_rows, w], F32, tag="ob")
                    ps_view = (psums[ci][:, :, :] if strided_rhs
                               else psums[ci][:, :, 1: 1 + w])
                    nc.scalar.mul(ob[:, :, :],
                                  ps_view,
                                  s_sb[:, ot: ot + 1])
                    nc.sync.dma_start(
                        out_d[img, ot * 128:(ot + 1) * 128,
                              ci * ch_rows:(ci + 1) * ch_rows, :],
                        ob[:, :, :])

    nc.compile()
    return nc


def quantize_weights(w):
    """Match reference fake-quant in f32: returns (q int-valued f32, scale)."""
    w = np.asarray(w, np.float32)
    amax = np.max(np.abs(w), axis=(1, 2, 3), keepdims=True).astype(np.float32)
    scale = np.maximum((amax / np.float32(QMAX)).astype(np.float32),
                       np.float32(SCALING_MIN_VAL)).astype(np.float32)
    q = np.clip(np.rint((w / scale).astype(np.float32)),
                -QMAX, QMAX).astype(np.float32)
    return q, scale.reshape(-1)


def pack_weights(q, o=O, c=C, np_dt=ml_dtypes.bfloat16):
    """q [O,C,3,3] int-valued -> [128, 9*KT*OT*128], [i,(tap,kt,ot,o)]."""
    KT = c // 128
    OT = o // 128
    w5 = q.reshape(OT, 128, KT, 128, KH, KW)       # [ot, ol, kt, i, kh, kw]
    w5 = w5.transpose(3, 4, 5, 2, 0, 1)            # [i, kh, kw, kt, ot, ol]
    w5 = np.ascontiguousarray(w5).reshape(128, 9 * KT * OT * 128)
    return w5.astype(np_dt)


_nc_cache = {}


def kernel(x, w):
    x = np.ascontiguousarray(np.asarray(x, np.float32))
    w = np.asarray(w, np.float32)
    assert x.shape == (N, C, H, W) and w.shape == (O, C, KH, KW)

    q, scale = quantize_weights(w)
    w_host = pack_weights(q)
    s_host = np.ascontiguousarray(
        scale.reshape(O // 128, 128).T).astype(np.float32)  # [o_local, ot]

    if "nc" not in _nc_cache:
        _nc_cache["nc"] = build_nc()
    nc = _nc_cache["nc"]

    in_maps = [
        {"x": np.ascontiguousarray(x[cid * NPC:(cid + 1) * NPC]),
         "wq": w_host, "scale": s_host}
        for cid in range(CORES)
    ]
    res = run_bass_kernel_spmd(nc, in_maps, list(range(CORES)))
    return np.concatenate([res.results[cid]["out"] for cid in range(CORES)],
                          axis=0)


if __name__ == "__main__":
    rng = np.random.default_rng(0)
    x = rng.standard_normal((N, C, H, W), dtype=np.float32)
    w = rng.standard_normal((O, C, KH, KW), dtype=np.float32) * 0.05
    out = kernel(x, w)
    print("out", out.shape, out.dtype, float(np.abs(out).max()))


# revision 8
# speedup vs baseline: 1.9389x; 1.0602x over previous
"""Trainium2 Bass kernel: Brevitas-style int4 fake-quant Conv2d (3x3, pad 1).

reference:
    wq = fake_quant_per_channel(w)          # per-O-channel int4 scale
    out = conv2d(x, wq, NCHW/OIHW, pad 1)

Strategy:
  * Host: per-channel abs-max quant -> integer weights q in [-7, 7]
    (exactly representable in bf16) + per-channel f32 scale.  The scale is
    applied on-device during the PSUM->SBUF drain, so the matmuls run on
    exact integer weights.
  * Device: data-parallel over batch (4 images per core x 8 cores).
    Conv lowered to implicit GEMM: for each of the 9 taps,
    out[o, p] += q[o, :, tap] @ x_shifted[:, p], accumulated in PSUM over
    (2 k-tiles x 9 taps x 2 passes).  x is stored zero-padded (58x58) in
    SBUF so every tap is a constant offset into a contiguous buffer.
  * Accuracy: x is split hi/lo into two bf16 tensors (hi = bf16(x),
    lo = bf16(x - hi)); both passes accumulate into the same PSUM bank.
    This carries ~16-17 mantissa bits of x through the PE array -> rel
    err ~1e-6 vs the f32 reference, at 2 bf16 matmul passes (half the
    cost of native fp32 matmul, which runs at 1/4 rate).
"""

import os
import sys
from contextlib import ExitStack

for _p in ("/opt/trn_rl_repo", "/root/.axon_site/_ro/trn_rl_repo"):
    if os.path.isdir(_p) and _p not in sys.path:
        sys.path.insert(0, _p)

import numpy as np
import ml_dtypes

import concourse.bass as bass  # noqa: F401  (bass types used via tile/bacc)
import concourse.mybir as mybir
import concourse.tile as tile
from concourse import bacc
from concourse.bass_utils import run_bass_kernel_spmd

F32 = mybir.dt.float32
BF16 = mybir.dt.bfloat16
FP16 = mybir.dt.float16

# Problem shapes (hardcoded per contract).
N, C, H, W = 32, 256, 56, 56
O, KH, KW = 256, 3, 3
CORES = 8
NPC = N // CORES  # images per core

QMAX = 7.0
SCALING_MIN_VAL = 2e-16


def build_nc(npc=NPC, c=C, h=H, w=W, o=O, two_pass=True, ch_rows=8,
             dt=BF16, strided_rhs=False, chunk_outer=False, strips=1):
    """Build the per-core Bass program (SPMD: same program on all cores).

    DRAM I/O (per core):
      x     [npc, c, h, w]  f32   batch shard
      wq    [128, 9*KT*OT*128] bf16  integer weights, layout [i, (tap,kt,ot,o)]
      scale [128, OT]       f32   per-out-channel scale, [o_local, ot]
      out   [npc, o, h, w]  f32
    """
    KT = c // 128
    OT = o // 128
    Hp, Wp = h + 2, w + 2
    PADN = Hp * Wp + 2          # +1 guard cell at each end
    assert h % ch_rows == 0
    n_chunks = h // ch_rows
    rhs_len = ch_rows * Wp
    assert rhs_len <= 512       # one PSUM bank (f32)

    nc = bacc.Bacc("TRN2", target_bir_lowering=False, debug=False)
    x_d = nc.dram_tensor("x", [npc, c, h, w], F32, kind="ExternalInput").ap()
    w_d = nc.dram_tensor("wq", [128, 9 * KT * OT * 128], dt,
                         kind="ExternalInput").ap()
    s_d = nc.dram_tensor("scale", [128, OT], F32, kind="ExternalInput").ap()
    out_d = nc.dram_tensor("out", [npc, o, h, w], F32,
                           kind="ExternalOutput").ap()

    n_passes = 2 if two_pass else 1

    with tile.TileContext(nc) as tc, ExitStack() as ctx:
        wpool = ctx.enter_context(tc.tile_pool(name="wpool", bufs=1))
        xpool = ctx.enter_context(tc.tile_pool(name="xpool", bufs=2))
        hpool = ctx.enter_context(tc.tile_pool(name="hpool", bufs=2))
        opool = ctx.enter_context(tc.tile_pool(name="opool", bufs=4))
        ppool = ctx.enter_context(tc.tile_pool(name="ppool",
                                               bufs=min(8, n_chunks + 1),
                                               space="PSUM"))

        w_sb = wpool.tile([128, 9 * KT * OT * 128], dt)
        nc.gpsimd.dma_start(w_sb[:, :], w_d[:, :])
        s_sb = wpool.tile([128, OT], F32)
        nc.gpsimd.dma_start(s_sb[:, :], s_d[:, :])

        def pad_views(t):
            """(interior [128,h,w] view, border memset APs) of padded tile."""
            interior = (t[:, Wp + 2: Wp + 2 + h * Wp]
                        .rearrange("p (r c) -> p r c", c=Wp)[:, :, 0:w])
            m1 = t[:, 0: Wp + 2]
            m2 = t[:, PADN - (Wp + 2): PADN]
            m3 = (t[:, 2 * Wp: 2 * Wp + (Hp - 3) * Wp]
                  .rearrange("p (a b) -> p a b", b=Wp)[:, :, 0:2])
            return interior, (m1, m2, m3)

        for img in range(npc):
            pads = []  # [kt] -> list of padded bf16 tiles (hi[, lo])
            assert h % strips == 0
            rs = h // strips  # rows per strip
            for kt in range(KT):
                xf = xpool.tile([128, h * w], F32, tag="xf")
                xv = xf[:, :].rearrange("p (r c) -> p r c", c=w)

                hi = hpool.tile([128, PADN], dt, tag=f"hi{kt}")
                hi_int, hi_borders = pad_views(hi)
                for m in hi_borders:
                    nc.vector.memset(m, 0.0)
                lo = None
                if two_pass:
                    lo = hpool.tile([128, PADN], dt, tag=f"lo{kt}")
                    lo_int, lo_borders = pad_views(lo)
                    for m in lo_borders:
                        nc.vector.memset(m, 0.0)
                for s in range(strips):
                    r0, r1 = s * rs, (s + 1) * rs
                    nc.sync.dma_start(
                        xv[:, r0:r1, :],
                        x_d[img, kt * 128:(kt + 1) * 128, r0:r1, :])
                    nc.vector.tensor_copy(hi_int[:, r0:r1, :], xv[:, r0:r1, :])
                    if two_pass:
                        # lo = dt(x - f32(hi)); exact f32 subtract on DVE
                        nc.vector.tensor_sub(lo_int[:, r0:r1, :],
                                             xv[:, r0:r1, :],
                                             hi_int[:, r0:r1, :])
                pads.append([hi, lo] if two_pass else [hi])

            psum_w = w if strided_rhs else Wp
            n_acc = KT * 9 * n_passes

            def mm_rhs(pt, ci, dh, dw):
                s0 = (ci * ch_rows + dh) * Wp + dw
                if strided_rhs:
                    return (pt[:, s0: s0 + rhs_len]
                            .rearrange("p (r c) -> p r c", c=Wp)[:, :, 0:w])
                return pt[:, s0: s0 + rhs_len]

            def drain(ps, ot, ci):
                ob = opool.tile([128, ch_rows, w], F32, tag="ob", name="ob")
                ps_view = (ps[:, :, :] if strided_rhs
                           else ps[:, :, 1: 1 + w])
                nc.scalar.mul(ob[:, :, :], ps_view, s_sb[:, ot: ot + 1])
                nc.sync.dma_start(
                    out_d[img, ot * 128:(ot + 1) * 128,
                          ci * ch_rows:(ci + 1) * ch_rows, :],
                    ob[:, :, :])

            def wslice(kt, tap, ot):
                j = ((tap * KT + kt) * OT + ot) * 128
                return w_sb[:, j: j + 128]

            if chunk_outer:
                for ot in range(OT):
                    for ci in range(n_chunks):
                        ps = ppool.tile([128, ch_rows, psum_w], F32,
                                        tag="ps", name=f"ps{ci}")
                        idx = 0
                        for kt in range(KT):
                            for tap in range(9):
                                dh, dw = divmod(tap, 3)
                                for pt in pads[kt]:
                                    nc.tensor.matmul(
                                        ps[:, :, :],
                                        wslice(kt, tap, ot),
                                        mm_rhs(pt, ci, dh, dw),
                                        start=(idx == 0),
                                        stop=(idx == n_acc - 1),
                                    )
                                    idx += 1
                        drain(ps, ot, ci)
            else:
                for ot in range(OT):
                    psums = [ppool.tile([128, ch_rows, psum_w], F32, tag="ps",
                                        name=f"ps{ci}")
                             for ci in range(n_chunks)]
                    idx = 0
                    for kt in range(KT):
                        for tap in range(9):
                            dh, dw = divmod(tap, 3)
                            for pt in pads[kt]:
                                for ci in range(n_chunks):
                                    nc.tensor.matmul(
                                        psums[ci][:, :, :],
                                        wslice(kt, tap, ot),
                                        mm_rhs(pt, ci, dh, dw),
                                        start=(idx == 0),
                                        stop=(idx == n_acc - 1),
                                    )
                                idx += 1
                    for ci in range(n_chunks):
                        drain(psums[ci], ot, ci)

    nc.compile()
    return nc


def quantize_weights(w):
    """Match reference fake-quant in f32: returns (q int-valued f32, scale)."""
    w = np.asarray(w, np.float32)
    amax = np.max(np.abs(w), axis=(1, 2, 3), keepdims=True).astype(np.float32)
    scale = np.maximum((amax / np.float32(QMAX)).astype(np.float32),
                       np.float32(SCALING_MIN_VAL)).astype(np.float32)
    q = np.clip(np.rint((w / scale).astype(np.float32)),
                -QMAX, QMAX).astype(np.float32)
    return q, scale.reshape(-1)


def pack_weights(q, o=O, c=C, np_dt=ml_dtypes.bfloat16):
    """q [O,C,3,3] int-valued -> [128, 9*KT*OT*128], [i,(tap,kt,ot,o)]."""
    KT = c // 128
    OT = o // 128
    w5 = q.reshape(OT, 128, KT, 128, KH, KW)       # [ot, ol, kt, i, kh, kw]
    w5 = w5.transpose(3, 4, 5, 2, 0, 1)            # [i, kh, kw, kt, ot, ol]
    w5 = np.ascontiguousarray(w5).reshape(128, 9 * KT * OT * 128)
    return w5.astype(np_dt)


_nc_cache = {}


def kernel(x, w):
    x = np.ascontiguousarray(np.asarray(x, np.float32))
    w = np.asarray(w, np.float32)
    assert x.shape == (N, C, H, W) and w.shape == (O, C, KH, KW)

    q, scale = quantize_weights(w)
    w_host = pack_weights(q)
    s_host = np.ascontiguousarray(
        scale.reshape(O // 128, 128).T).astype(np.float32)  # [o_local, ot]

    if "nc" not in _nc_cache:
        _nc_cache["nc"] = build_nc()
    nc = _nc_cache["nc"]

    in_maps = [
        {"x": np.ascontiguousarray(x[cid * NPC:(cid + 1) * NPC]),
         "wq": w_host, "scale": s_host}
        for cid in range(CORES)
    ]
    res = run_bass_kernel_spmd(nc, in_maps, list(range(CORES)))
    return np.concatenate([res.results[cid]["out"] for cid in range(CORES)],
                          axis=0)


if __name__ == "__main__":
    rng = np.random.default_rng(0)
    x = rng.standard_normal((N, C, H, W), dtype=np.float32)
    w = rng.standard_normal((O, C, KH, KW), dtype=np.float32) * 0.05
    out = kernel(x, w)
    print("out", out.shape, out.dtype, float(np.abs(out).max()))


# revision 9
# speedup vs baseline: 2.0148x; 1.0392x over previous
"""Trainium2 Bass kernel: Brevitas-style int4 fake-quant Conv2d (3x3, pad 1).

reference:
    wq = fake_quant_per_channel(w)          # per-O-channel int4 scale
    out = conv2d(x, wq, NCHW/OIHW, pad 1)

Strategy:
  * Host: per-channel abs-max quant -> integer weights q in [-7, 7]
    (exactly representable in bf16) + per-channel f32 scale.  The scale is
    applied on-device during the PSUM->SBUF drain, so the matmuls run on
    exact integer weights.
  * Device: data-parallel over batch (4 images per core x 8 cores).
    Conv lowered to implicit GEMM: for each of the 9 taps,
    out[o, p] += q[o, :, tap] @ x_shifted[:, p], accumulated in PSUM over
    (2 k-tiles x 9 taps x 2 passes).  x is stored zero-padded (58x58) in
    SBUF so every tap is a constant offset into a contiguous buffer.
  * Accuracy: x is split hi/lo into two bf16 tensors (hi = bf16(x),
    lo = bf16(x - hi)); both passes accumulate into the same PSUM bank.
    This carries ~16-17 mantissa bits of x through the PE array -> rel
    err ~1e-6 vs the f32 reference, at 2 bf16 matmul passes (half the
    cost of native fp32 matmul, which runs at 1/4 rate).
"""

import os
import sys
from contextlib import ExitStack

for _p in ("/opt/trn_rl_repo", "/root/.axon_site/_ro/trn_rl_repo"):
    if os.path.isdir(_p) and _p not in sys.path:
        sys.path.insert(0, _p)

import numpy as np
import ml_dtypes

import concourse.bass as bass  # noqa: F401  (bass types used via tile/bacc)
import concourse.mybir as mybir
import concourse.tile as tile
from concourse import bacc
from concourse.bass_utils import run_bass_kernel_spmd

F32 = mybir.dt.float32
BF16 = mybir.dt.bfloat16
FP16 = mybir.dt.float16
FP8 = mybir.dt.float8e4

# Problem shapes (hardcoded per contract).
N, C, H, W = 32, 256, 56, 56
O, KH, KW = 256, 3, 3
CORES = 8
NPC = N // CORES  # images per core

QMAX = 7.0
SCALING_MIN_VAL = 2e-16


def build_nc(npc=NPC, c=C, h=H, w=W, o=O, two_pass=False, ch_rows=8,
             dt=FP16, w_dt=FP8, chunk_outer=True, strips=7):
    """Build the per-core Bass program (SPMD: same program on all cores).

    DRAM I/O (per core):
      x     [npc, c, h, w]  f32   batch shard
      wq    [128, 9*KT*OT*128] w_dt  integer weights, layout [i,(tap,kt,ot,o)]
      scale [128, OT]       f32   per-out-channel scale, [o_local, ot]
      out   [npc, o, h, w]  f32
    """
    KT = c // 128
    OT = o // 128
    Hp, Wp = h + 2, w + 2
    assert h % ch_rows == 0 and h % strips == 0
    n_chunks = h // ch_rows
    rs = h // strips
    assert ch_rows * w <= 512        # one PSUM bank (f32)

    nc = bacc.Bacc("TRN2", target_bir_lowering=False, debug=False)
    x_d = nc.dram_tensor("x", [npc, c, h, w], F32, kind="ExternalInput").ap()
    w_d = nc.dram_tensor("wq", [128, 9 * KT * OT * 128], w_dt,
                         kind="ExternalInput").ap()
    s_d = nc.dram_tensor("scale", [128, OT], F32, kind="ExternalInput").ap()
    out_d = nc.dram_tensor("out", [npc, o, h, w], F32,
                           kind="ExternalOutput").ap()

    n_passes = 2 if two_pass else 1
    n_acc = KT * 9 * n_passes

    with tile.TileContext(nc) as tc, ExitStack() as ctx:
        wpool = ctx.enter_context(tc.tile_pool(name="wpool", bufs=1))
        xpool = ctx.enter_context(tc.tile_pool(name="xpool", bufs=2))
        hpool = ctx.enter_context(tc.tile_pool(name="hpool", bufs=2))
        opool = ctx.enter_context(tc.tile_pool(name="opool", bufs=4))
        ppool = ctx.enter_context(tc.tile_pool(name="ppool",
                                               bufs=min(8, n_chunks + 1),
                                               space="PSUM"))

        # Weights go first on the main DMA queue: the very first matmul
        # depends on them, so they must not queue behind the x loads.
        w_sb = wpool.tile([128, 9 * KT * OT * 128], w_dt)
        nc.sync.dma_start(w_sb[:, :], w_d[:, :])
        s_sb = wpool.tile([128, OT], F32)
        nc.gpsimd.dma_start(s_sb[:, :], s_d[:, :])

        def border_memsets(t):
            """Zero rows 0/Hp-1 and cols 0/Wp-1 of a [128, Hp*Wp] tile."""
            yield t[:, 0: Wp + 1]                              # row 0 + (1,0)
            yield t[:, (Hp - 1) * Wp - 1: Hp * Wp]             # (Hp-2,Wp-1)+row
            yield (t[:, 2 * Wp - 1: 2 * Wp - 1 + (Hp - 3) * Wp]
                   .rearrange("p (a b) -> p a b", b=Wp)[:, :, 0:2])

        for img in range(npc):
            pads = []  # [kt] -> list of padded-view [128, Hp, Wp] APs
            for kt in range(KT):
                xf = xpool.tile([128, h * w], F32, tag="xf")
                xv = xf[:, :].rearrange("p (r c) -> p r c", c=w)

                hi = hpool.tile([128, Hp * Wp], dt, tag=f"hi{kt}")
                for m in border_memsets(hi):
                    nc.vector.memset(m, 0.0)
                hiv = hi[:, :].rearrange("p (r c) -> p r c", c=Wp)
                views = [hiv]
                lov = None
                if two_pass:
                    lo = hpool.tile([128, Hp * Wp], dt, tag=f"lo{kt}")
                    for m in border_memsets(lo):
                        nc.vector.memset(m, 0.0)
                    lov = lo[:, :].rearrange("p (r c) -> p r c", c=Wp)
                    views.append(lov)
                for s in range(strips):
                    r0, r1 = s * rs, (s + 1) * rs
                    nc.sync.dma_start(
                        xv[:, r0:r1, :],
                        x_d[img, kt * 128:(kt + 1) * 128, r0:r1, :])
                    nc.vector.tensor_copy(hiv[:, r0 + 1:r1 + 1, 1:1 + w],
                                          xv[:, r0:r1, :])
                    if two_pass:
                        # lo = dt(x - f32(hi)); exact f32 subtract on DVE
                        nc.vector.tensor_sub(lov[:, r0 + 1:r1 + 1, 1:1 + w],
                                             xv[:, r0:r1, :],
                                             hiv[:, r0 + 1:r1 + 1, 1:1 + w])
                pads.append(views)

            def wslice(kt, tap, ot):
                j = ((tap * KT + kt) * OT + ot) * 128
                return w_sb[:, j: j + 128]

            def drain(ps, ot, ci):
                ob = opool.tile([128, ch_rows, w], F32, tag="ob", name="ob")
                nc.scalar.mul(ob[:, :, :], ps[:, :, :], s_sb[:, ot: ot + 1])
                nc.sync.dma_start(
                    out_d[img, ot * 128:(ot + 1) * 128,
                          ci * ch_rows:(ci + 1) * ch_rows, :],
                    ob[:, :, :])

            def acc_mms(ps, ot, ci):
                idx = 0
                for kt in range(KT):
                    for tap in range(9):
                        dh, dw = divmod(tap, 3)
                        for pt in pads[kt]:
                            nc.tensor.matmul(
                                ps[:, :, :],
                                wslice(kt, tap, ot),
                                pt[:, ci * ch_rows + dh:
                                   ci * ch_rows + dh + ch_rows,
                                   dw: dw + w],
                                start=(idx == 0),
                                stop=(idx == n_acc - 1),
                            )
                            idx += 1

            if chunk_outer:
                for ot in range(OT):
                    for ci in range(n_chunks):
                        ps = ppool.tile([128, ch_rows, w], F32,
                                        tag="ps", name=f"ps{ci}")
                        acc_mms(ps, ot, ci)
                        drain(ps, ot, ci)
            else:
                for ot in range(OT):
                    psums = [ppool.tile([128, ch_rows, w], F32, tag="ps",
                                        name=f"ps{ci}")
                             for ci in range(n_chunks)]
                    idx = 0
                    for kt in range(KT):
                        for tap in range(9):
                            dh, dw = divmod(tap, 3)
                            for pt in pads[kt]:
                                for ci in range(n_chunks):
                                    nc.tensor.matmul(
                                        psums[ci][:, :, :],
                                        wslice(kt, tap, ot),
                                        pt[:, ci * ch_rows + dh:
                                           ci * ch_rows + dh + ch_rows,
                                           dw: dw + w],
                                        start=(idx == 0),
                                        stop=(idx == n_acc - 1),
                                    )
                                idx += 1
                    for ci in range(n_chunks):
                        drain(psums[ci], ot, ci)

    nc.compile()
    return nc


def quantize_weights(w):
    """Match reference fake-quant in f32: returns (q int-valued f32, scale)."""
    w = np.asarray(w, np.float32)
    amax = np.max(np.abs(w), axis=(1, 2, 3), keepdims=True).astype(np.float32)
    scale = np.maximum((amax / np.float32(QMAX)).astype(np.float32),
                       np.float32(SCALING_MIN_VAL)).astype(np.float32)
    q = np.clip(np.rint((w / scale).astype(np.float32)),
                -QMAX, QMAX).astype(np.float32)
    return q, scale.reshape(-1)


def pack_weights(q, o=O, c=C, np_dt=ml_dtypes.float8_e4m3):
    """q [O,C,3,3] int-valued -> [128, 9*KT*OT*128], [i,(tap,kt,ot,o)]."""
    KT = c // 128
    OT = o // 128
    w5 = q.reshape(OT, 128, KT, 128, KH, KW)       # [ot, ol, kt, i, kh, kw]
    w5 = w5.transpose(3, 4, 5, 2, 0, 1)            # [i, kh, kw, kt, ot, ol]
    w5 = np.ascontiguousarray(w5).reshape(128, 9 * KT * OT * 128)
    return w5.astype(np_dt)


_nc_cache = {}


def kernel(x, w):
    x = np.ascontiguousarray(np.asarray(x, np.float32))
    w = np.asarray(w, np.float32)
    assert x.shape == (N, C, H, W) and w.shape == (O, C, KH, KW)

    q, scale = quantize_weights(w)
    w_host = pack_weights(q)
    s_host = np.ascontiguousarray(
        scale.reshape(O // 128, 128).T).astype(np.float32)  # [o_local, ot]

    if "nc" not in _nc_cache:
        _nc_cache["nc"] = build_nc()
    nc = _nc_cache["nc"]

    in_maps = [
        {"x": np.ascontiguousarray(x[cid * NPC:(cid + 1) * NPC]),
         "wq": w_host, "scale": s_host}
        for cid in range(CORES)
    ]
    res = run_bass_kernel_spmd(nc, in_maps, list(range(CORES)))
    return np.concatenate([res.results[cid]["out"] for cid in range(CORES)],
                          axis=0)


if __name__ == "__main__":
    rng = np.random.default_rng(0)
    x = rng.standard_normal((N, C, H, W), dtype=np.float32)
    w = rng.standard_normal((O, C, KH, KW), dtype=np.float32) * 0.05
    out = kernel(x, w)
    print("out", out.shape, out.dtype, float(np.abs(out).max()))


# revision 11
# speedup vs baseline: 2.0186x; 1.0019x over previous
"""Trainium2 Bass kernel: Brevitas-style int4 fake-quant Conv2d (3x3, pad 1).

reference:
    wq = fake_quant_per_channel(w)          # per-O-channel int4 scale
    out = conv2d(x, wq, NCHW/OIHW, pad 1)

Strategy (measured ~215 us per-core span on trn2, rel err ~2e-4):
  * Host: per-channel abs-max quant -> integer weights q in [-7, 7]
    (exactly representable in fp8 e4m3) + per-channel f32 scale.  The
    scale is applied on-device during the PSUM->SBUF drain
    (scalar-engine activation with a per-partition [128,1] scale), so
    the matmuls run on exact integer weights.
  * Device: data-parallel over batch (4 images per core x 8 cores).
    Conv lowered to implicit GEMM: for each of the 9 taps,
    out[o, p] += q[o, :, tap] @ x_shifted[:, p], accumulated in PSUM
    over 2 k-tiles x 9 taps.  x is cast f32->fp16 into a zero-padded
    58x58 SBUF view, so every tap is a [8 rows, 56 cols] strided slice
    of one buffer (stride 58); output chunks of 8 rows = 448 f32 fit
    one PSUM bank.  Chunk-outer accumulation keeps the PE at zero-gap
    back-to-back matmuls (1008 MMs x ~193 ns) while drains/DMA overlap;
    the first image's x load is split into 7 row-strips so the first
    matmul starts as soon as strip 0 lands (Tile deps are range-aware).
  * Accuracy: fp16 carries 11 mantissa bits of x, weights are exact,
    PSUM accumulates in f32 -> absmax rel err ~2e-4 vs the f32
    reference.  (A two_pass=True build splits x hi/lo into two fp16
    passes -> ~4e-7, at 2x the PE cost, if ever needed.)
"""

import os
import sys
from contextlib import ExitStack

for _p in ("/opt/trn_rl_repo", "/root/.axon_site/_ro/trn_rl_repo"):
    if os.path.isdir(_p) and _p not in sys.path:
        sys.path.insert(0, _p)

import numpy as np
import ml_dtypes

import concourse.bass as bass  # noqa: F401  (bass types used via tile/bacc)
import concourse.mybir as mybir
import concourse.tile as tile
from concourse import bacc
from concourse.bass_utils import run_bass_kernel_spmd

F32 = mybir.dt.float32
BF16 = mybir.dt.bfloat16
FP16 = mybir.dt.float16
FP8 = mybir.dt.float8e4

# Problem shapes (hardcoded per contract).
N, C, H, W = 32, 256, 56, 56
O, KH, KW = 256, 3, 3
CORES = 8
NPC = N // CORES  # images per core

QMAX = 7.0
SCALING_MIN_VAL = 2e-16


def build_nc(npc=NPC, c=C, h=H, w=W, o=O, two_pass=False, ch_rows=8,
             dt=FP16, w_dt=FP8, chunk_outer=True, strips=7):
    """Build the per-core Bass program (SPMD: same program on all cores).

    DRAM I/O (per core):
      x     [npc, c, h, w]  f32   batch shard
      wq    [128, 9*KT*OT*128] w_dt  integer weights, layout [i,(tap,kt,ot,o)]
      scale [128, OT]       f32   per-out-channel scale, [o_local, ot]
      out   [npc, o, h, w]  f32
    """
    KT = c // 128
    OT = o // 128
    Hp, Wp = h + 2, w + 2
    assert h % ch_rows == 0 and h % strips == 0
    n_chunks = h // ch_rows
    rs = h // strips
    assert ch_rows * w <= 512        # one PSUM bank (f32)

    nc = bacc.Bacc("TRN2", target_bir_lowering=False, debug=False)
    x_d = nc.dram_tensor("x", [npc, c, h, w], F32, kind="ExternalInput").ap()
    w_d = nc.dram_tensor("wq", [128, 9 * KT * OT * 128], w_dt,
                         kind="ExternalInput").ap()
    s_d = nc.dram_tensor("scale", [128, OT], F32, kind="ExternalInput").ap()
    out_d = nc.dram_tensor("out", [npc, o, h, w], F32,
                           kind="ExternalOutput").ap()

    n_passes = 2 if two_pass else 1
    n_acc = KT * 9 * n_passes

    with tile.TileContext(nc) as tc, ExitStack() as ctx:
        wpool = ctx.enter_context(tc.tile_pool(name="wpool", bufs=1))
        xpool = ctx.enter_context(tc.tile_pool(name="xpool", bufs=2))
        hpool = ctx.enter_context(tc.tile_pool(name="hpool", bufs=2))
        opool = ctx.enter_context(tc.tile_pool(name="opool", bufs=4))
        ppool = ctx.enter_context(tc.tile_pool(name="ppool",
                                               bufs=min(8, n_chunks + 1),
                                               space="PSUM"))

        # Weights go first on the main DMA queue: the very first matmul
        # depends on them, so they must not queue behind the x loads.
        w_sb = wpool.tile([128, 9 * KT * OT * 128], w_dt)
        nc.sync.dma_start(w_sb[:, :], w_d[:, :])
        s_sb = wpool.tile([128, OT], F32)
        nc.gpsimd.dma_start(s_sb[:, :], s_d[:, :])

        def border_memsets(t):
            """Zero rows 0/Hp-1 and cols 0/Wp-1 of a [128, Hp*Wp] tile."""
            yield t[:, 0: Wp + 1]                              # row 0 + (1,0)
            yield t[:, (Hp - 1) * Wp - 1: Hp * Wp]             # (Hp-2,Wp-1)+row
            yield (t[:, 2 * Wp - 1: 2 * Wp - 1 + (Hp - 3) * Wp]
                   .rearrange("p (a b) -> p a b", b=Wp)[:, :, 0:2])

        for img in range(npc):
            pads = []  # [kt] -> list of padded-view [128, Hp, Wp] APs
            for kt in range(KT):
                xf = xpool.tile([128, h * w], F32, tag="xf")
                xv = xf[:, :].rearrange("p (r c) -> p r c", c=w)

                hi = hpool.tile([128, Hp * Wp], dt, tag=f"hi{kt}")
                for m in border_memsets(hi):
                    nc.vector.memset(m, 0.0)
                hiv = hi[:, :].rearrange("p (r c) -> p r c", c=Wp)
                views = [hiv]
                lov = None
                if two_pass:
                    lo = hpool.tile([128, Hp * Wp], dt, tag=f"lo{kt}")
                    for m in border_memsets(lo):
                        nc.vector.memset(m, 0.0)
                    lov = lo[:, :].rearrange("p (r c) -> p r c", c=Wp)
                    views.append(lov)
                for s in range(strips):
                    r0, r1 = s * rs, (s + 1) * rs
                    nc.sync.dma_start(
                        xv[:, r0:r1, :],
                        x_d[img, kt * 128:(kt + 1) * 128, r0:r1, :])
                    nc.vector.tensor_copy(hiv[:, r0 + 1:r1 + 1, 1:1 + w],
                                          xv[:, r0:r1, :])
                    if two_pass:
                        # lo = dt(x - f32(hi)); exact f32 subtract on DVE
                        nc.vector.tensor_sub(lov[:, r0 + 1:r1 + 1, 1:1 + w],
                                             xv[:, r0:r1, :],
                                             hiv[:, r0 + 1:r1 + 1, 1:1 + w])
                pads.append(views)

            def wslice(kt, tap, ot):
                j = ((tap * KT + kt) * OT + ot) * 128
                return w_sb[:, j: j + 128]

            def drain(ps, ot, ci):
                ob = opool.tile([128, ch_rows, w], F32, tag="ob", name="ob")
                nc.scalar.mul(ob[:, :, :], ps[:, :, :], s_sb[:, ot: ot + 1])
                nc.sync.dma_start(
                    out_d[img, ot * 128:(ot + 1) * 128,
                          ci * ch_rows:(ci + 1) * ch_rows, :],
                    ob[:, :, :])

            def acc_mms(ps, ot, ci):
                idx = 0
                for kt in range(KT):
                    for tap in range(9):
                        dh, dw = divmod(tap, 3)
                        for pt in pads[kt]:
                            nc.tensor.matmul(
                                ps[:, :, :],
                                wslice(kt, tap, ot),
                                pt[:, ci * ch_rows + dh:
                                   ci * ch_rows + dh + ch_rows,
                                   dw: dw + w],
                                start=(idx == 0),
                                stop=(idx == n_acc - 1),
                            )
                            idx += 1

            if chunk_outer:
                for ot in range(OT):
                    for ci in range(n_chunks):
                        ps = ppool.tile([128, ch_rows, w], F32,
                                        tag="ps", name=f"ps{ci}")
                        acc_mms(ps, ot, ci)
                        drain(ps, ot, ci)
            else:
                for ot in range(OT):
                    psums = [ppool.tile([128, ch_rows, w], F32, tag="ps",
                                        name=f"ps{ci}")
                             for ci in range(n_chunks)]
                    idx = 0
                    for kt in range(KT):
                        for tap in range(9):
                            dh, dw = divmod(tap, 3)
                            for pt in pads[kt]:
                                for ci in range(n_chunks):
                                    nc.tensor.matmul(
                                        psums[ci][:, :, :],
                                        wslice(kt, tap, ot),
                                        pt[:, ci * ch_rows + dh:
                                           ci * ch_rows + dh + ch_rows,
                                           dw: dw + w],
                                        start=(idx == 0),
                                        stop=(idx == n_acc - 1),
                                    )
                                idx += 1
                    for ci in range(n_chunks):
                        drain(psums[ci], ot, ci)

    nc.compile()
    return nc


def quantize_weights(w):
    """Match reference fake-quant in f32: returns (q int-valued f32, scale)."""
    w = np.asarray(w, np.float32)
    amax = np.max(np.abs(w), axis=(1, 2, 3), keepdims=True).astype(np.float32)
    scale = np.maximum((amax / np.float32(QMAX)).astype(np.float32),
                       np.float32(SCALING_MIN_VAL)).astype(np.float32)
    q = np.clip(np.rint((w / scale).astype(np.float32)),
                -QMAX, QMAX).astype(np.float32)
    return q, scale.reshape(-1)


def pack_weights(q, o=O, c=C, np_dt=ml_dtypes.float8_e4m3):
    """q [O,C,3,3] int-valued -> [128, 9*KT*OT*128], [i,(tap,kt,ot,o)]."""
    KT = c // 128
    OT = o // 128
    w5 = q.reshape(OT, 128, KT, 128, KH, KW)       # [ot, ol, kt, i, kh, kw]
    w5 = w5.transpose(3, 4, 5, 2, 0, 1)            # [i, kh, kw, kt, ot, ol]
    w5 = np.ascontiguousarray(w5).reshape(128, 9 * KT * OT * 128)
    return w5.astype(np_dt)


_nc_cache = {}
LAST_RESULT = None  # BassKernelResults of the most recent kernel() call


def kernel(x, w):
    global LAST_RESULT
    x = np.ascontiguousarray(np.asarray(x, np.float32))
    w = np.asarray(w, np.float32)
    assert x.shape == (N, C, H, W) and w.shape == (O, C, KH, KW)

    q, scale = quantize_weights(w)
    w_host = pack_weights(q)
    s_host = np.ascontiguousarray(
        scale.reshape(O // 128, 128).T).astype(np.float32)  # [o_local, ot]

    if "nc" not in _nc_cache:
        _nc_cache["nc"] = build_nc()
    nc = _nc_cache["nc"]

    in_maps = [
        {"x": np.ascontiguousarray(x[cid * NPC:(cid + 1) * NPC]),
         "wq": w_host, "scale": s_host}
        for cid in range(CORES)
    ]
    kwargs = {}
    trace_dir = os.environ.get("KERNEL_TRACE_DIR")
    if trace_dir:  # dev-harness profiling only; unset in normal use
        kwargs = {"trace": True, "tmpdir": trace_dir}
    res = run_bass_kernel_spmd(nc, in_maps, list(range(CORES)), **kwargs)
    LAST_RESULT = res
    return np.concatenate([res.results[cid]["out"] for cid in range(CORES)],
                          axis=0)


if __name__ == "__main__":
    rng = np.random.default_rng(0)
    x = rng.standard_normal((N, C, H, W), dtype=np.float32)
    w = rng.standard_normal((O, C, KH, KW), dtype=np.float32) * 0.05
    out = kernel(x, w)
    print("out", out.shape, out.dtype, float(np.abs(out).max()))
